# revision 36
# baseline (speedup 1.0000x reference)
"""BFP-quantized linear layer (BFLinear) for Trainium2, 8-core data-parallel.

Computes: out = bfp_q(x, 8, 16) @ bfp_q(w, 8, 16).T + bias
  where bfp_q groups 16 contiguous elements along the feature axis, shares
  exponent e = floor(log2(max|g|)), rounds mantissas to `bit` bits (RNE) and
  clips to [-2^(bit-1), 2^(bit-1)-1].

Math on-device (bit-exact vs the jax reference, up to matmul accumulation):
  gmax  = max|group|                       (DVE reduce, abs)
  gmc   = max(gmax, FLT_MIN)
  recipB= 2^(bit-1-e)  (bit tricks: ((bits&EM)^EM) + (bit-2)<<23)
  sca   = 2^(e-(bit-1)) = (bits&EM as float) * 2^-(bit-1)
  v     = x * recipB                       (TT, exact pow2 scaling, Pool)
  u     = clamp(v, lo, hi)                 (TS min/max, Pool)
  t     = (u + C) + (-C) -> bf16 ints      (TS add/add 2x on DVE; C=1.5*2^23
                                            forces RNE at integer granularity)
  xq    = t * bf16(sca)                    (TT bf16, exact, DVE)
Then out = xq @ wq.T + bias via bf16 TensorE matmuls accumulated in fp32 PSUM;
bias seeded into PSUM by a K=2 bf16 matmul (ones x [bias_hi; bias_lo]).
Output is written bf16 (error <= 2^-9 relative, well within tolerance) and
upcast to f32 on host, halving output HBM traffic.

Sharding: rows of x split evenly across 8 NeuronCores; weight/bias replicated.
Quantization groups lie along K (feature) so row sharding never splits one.

Scheduling: engine queues execute in order, so the emission is software-
pipelined — input DMA runs two chunks ahead and the reduce/smalls/mult/clamp
stage one chunk ahead of the round/scale stage; this keeps the DVE from
head-of-line blocking on the Pool's clamp and keeps PE continuously fed
(holding its fast p-state).

Hardware op-shape rules learned from traces (violating any costs 2-25x):
  - scalar_tensor_tensor is DVE-only; tensor_scalar on Pool only with min/max;
    no subtract-immediates or negative int immediates anywhere on TS;
    f32->bf16 TS writes only via the (add,add) dual; casts on ACT, not DVE.
"""

import os
import sys

import numpy as np

for _p in ("/opt/trn_rl_repo",):
    if _p not in sys.path and os.path.isdir(_p):
        sys.path.append(_p)

N_CORES = 8

# engine per stage: 'v' DVE, 'g' GPSIMD/Pool, 'a' ACT/scalar
ENG_CFG = {
    "reduce": "v",      # gmax group absmax (DVE only: gpsimd can't reduce X)
    "smalls": "v",      # [P, G] bit-trick ops
    "mult": "v",        # v = x * recipB  (TT f32; Pool TT is 2x slower and
                        # contends with DVE on the shared SBUF port)
    "clamp": "g",       # u = clamp(v, lo, hi)  (TS min/max — Pool-proven)
    "round": "v",       # t = (u + C) + -C -> bf16  (TS add/add, 2x on DVE)
    "scalemult": "v",   # xq = t * scab  (TT bf16, DVE)
    "xqtcopy": "a",     # PSUM->SBUF copy of transposed xq
    "outcopy": "a",     # PSUM->SBUF copy of out (f32->bf16)
}

_CACHE = {}


def _eng(nc, which, idx=0):
    s = {"v": nc.vector, "g": nc.gpsimd, "a": nc.scalar}
    return s[which[idx % len(which)]]


def _bcast_group_ap(t, G, sz):
    """AP reading tile t[P, G] as [P, G, sz] with the last dim broadcast."""
    import concourse.bass as bass

    ap = t.ap.copy()
    ap.append([0, sz])
    return bass.AP(tensor=t.tensor, offset=t.offset, ap=ap)


def _quant_a(nc, pools, xt, F, bit, sz, cfg, ci=0):
    """Stage A: group stats + normalize + clamp. Returns (u, scab)."""
    import concourse.mybir as mybir

    f32 = mybir.dt.float32
    i32 = mybir.dt.int32
    bf16 = mybir.dt.bfloat16
    P = 128
    G = F // sz
    qmax = float(2 ** (bit - 1) - 1)
    hi = float(np.nextafter(np.float32(qmax + 0.5), np.float32(0.0)))
    lo = float(np.nextafter(np.float32(-qmax - 1.5), np.float32(0.0)))
    FLT_MIN = float(2.0**-119)
    EXPMASK = 0x7F800000

    er = _eng(nc, cfg["reduce"], ci)
    es = _eng(nc, cfg["smalls"], ci)
    em = _eng(nc, cfg["mult"], ci)
    eu = _eng(nc, cfg["clamp"], ci)

    gmax = pools["sml"].tile([P, G], f32, tag="gmax")
    er.tensor_reduce(
        out=gmax,
        in_=xt.rearrange("p (g s) -> p g s", s=sz),
        axis=mybir.AxisListType.X,
        op=mybir.AluOpType.max,
        apply_absolute_value=True,
    )
    gmc = pools["sml"].tile([P, G], f32, tag="gmc")
    es.tensor_scalar(
        out=gmc, in0=gmax, scalar1=FLT_MIN, scalar2=None, op0=mybir.AluOpType.max
    )
    recip = pools["sml"].tile([P, G], i32, tag="recip")
    es.tensor_scalar(
        out=recip,
        in0=gmc.bitcast(i32),
        scalar1=EXPMASK,
        scalar2=EXPMASK,
        op0=mybir.AluOpType.bitwise_and,
        op1=mybir.AluOpType.bitwise_xor,
    )
    recipB = pools["sml"].tile([P, G], i32, tag="recipB")
    es.tensor_scalar(
        out=recipB,
        in0=recip,
        scalar1=(bit - 2) << 23,
        scalar2=None,
        op0=mybir.AluOpType.add,
    )
    pow2e = pools["sml"].tile([P, G], i32, tag="pow2e")
    es.tensor_scalar(
        out=pow2e,
        in0=gmc.bitcast(i32),
        scalar1=EXPMASK,
        scalar2=None,
        op0=mybir.AluOpType.bitwise_and,
    )
    # sca = pow2e * 2^-(bit-1) as float mult (exact; int subtract / negative
    # int-add immediates fall off the DVE fast path: 3.6-4.4us for [128,128])
    sca = pools["sml"].tile([P, G], f32, tag="sca")
    es.tensor_scalar(
        out=sca,
        in0=pow2e.bitcast(f32),
        scalar1=float(2.0 ** (-(bit - 1))),
        scalar2=None,
        op0=mybir.AluOpType.mult,
    )
    # f32->bf16 cast: ~300ns on ACT vs ~1us CAST on DVE
    scab = pools["sml"].tile([P, G], bf16, tag="scab")
    nc.scalar.copy(scab, sca)

    v = pools["v"].tile([P, F], f32, tag="v")
    em.tensor_tensor(
        out=v,
        in0=xt,
        in1=_bcast_group_ap(recipB.bitcast(f32), G, sz),
        op=mybir.AluOpType.mult,
    )
    u = pools["u"].tile([P, F], f32, tag="u")
    eu.tensor_scalar(
        out=u,
        in0=v,
        scalar1=hi,
        scalar2=lo,
        op0=mybir.AluOpType.min,
        op1=mybir.AluOpType.max,
    )
    return u, scab


def _quant_b(nc, pools, u, scab, F, bit, sz, out_bf16, cfg, ci=0):
    """Stage B: RNE round to bf16 ints, then exact scale-back (bf16 TT)."""
    import concourse.mybir as mybir

    bf16 = mybir.dt.bfloat16
    P = 128
    G = F // sz
    C = float(np.float32(1.5 * 2.0**23))

    et = _eng(nc, cfg["round"], ci)
    ex = _eng(nc, cfg["scalemult"], ci)

    t = pools["t"].tile([P, F], bf16, tag="t")
    et.tensor_scalar(
        out=t,
        in0=u,
        scalar1=C,
        scalar2=-C,
        op0=mybir.AluOpType.add,
        op1=mybir.AluOpType.add,
    )
    ex.tensor_tensor(
        out=out_bf16,
        in0=t,
        in1=_bcast_group_ap(scab, G, sz),
        op=mybir.AluOpType.mult,
    )


def _build(nrows, K, O, x_bit, w_bit, x_sz, w_sz, cfg=None):
    import concourse.bacc as bacc
    import concourse.bass as bass  # noqa: F401
    import concourse.mybir as mybir
    import concourse.tile as tile
    from concourse.masks import make_identity

    cfg = dict(ENG_CFG, **(cfg or {}))
    f32 = mybir.dt.float32
    bf16 = mybir.dt.bfloat16

    P = 128
    RPC = 1024  # rows per chunk
    assert nrows % RPC == 0
    n_chunks = nrows // RPC
    FB = RPC // P  # row-blocks per chunk (8)
    F = FB * K  # free columns per chunk
    KC = K // P  # k-chunks (4)
    OB = O // P  # o-blocks (4)

    nc = bacc.Bacc("TRN2", debug=False)
    x_d = nc.dram_tensor("x", (nrows, K), f32, kind="ExternalInput").ap()
    w_d = nc.dram_tensor("w", (O, K), f32, kind="ExternalInput").ap()
    b_d = nc.dram_tensor("b", (1, O), f32, kind="ExternalInput").ap()
    o_d = nc.dram_tensor("out", (nrows, O), bf16, kind="ExternalOutput").ap()

    with tile.TileContext(nc) as tc:
        with (
            tc.tile_pool(name="const", bufs=1) as constp,
            tc.tile_pool(name="wsb", bufs=1) as wsb,
            tc.tile_pool(name="xraw", bufs=2) as xraw,
            tc.tile_pool(name="sml", bufs=4) as sml,
            tc.tile_pool(name="v", bufs=2) as vp,
            tc.tile_pool(name="u", bufs=2) as up,
            tc.tile_pool(name="t", bufs=2) as tp,
            tc.tile_pool(name="xq", bufs=3) as xqp,
            tc.tile_pool(name="xqT", bufs=4) as xqTp,
            tc.tile_pool(name="osb", bufs=4) as osb,
            tc.tile_pool(name="psT", bufs=2, space="PSUM") as psT,
            tc.tile_pool(name="psO", bufs=3, space="PSUM") as psO,
        ):
            pools = {"sml": sml, "v": vp, "u": up, "t": tp}

            ident = constp.tile([P, P], bf16)
            make_identity(nc, ident)
            ones2 = constp.tile([2, P], bf16)
            nc.vector.memset(ones2, 1.0)
            bias_sb = constp.tile([1, O], f32)
            nc.sync.dma_start(out=bias_sb, in_=b_d)
            # bias split into bf16 hi + lo so a K=2 bf16 matmul seeds PSUM
            # with fp32-accurate bias (error ~2^-17 relative)
            bhi = constp.tile([1, O], bf16)
            nc.vector.tensor_copy(out=bhi, in_=bias_sb)
            bhi32 = constp.tile([1, O], f32)
            nc.vector.tensor_copy(out=bhi32, in_=bhi)
            blo32 = constp.tile([1, O], f32)
            nc.vector.tensor_tensor(
                out=blo32, in0=bias_sb, in1=bhi32, op=mybir.AluOpType.subtract
            )
            blo = constp.tile([1, O], bf16)
            nc.vector.tensor_copy(out=blo, in_=blo32)
            brow = constp.tile([2, O], bf16)
            nc.sync.dma_start(out=brow[0:1, :], in_=bhi)
            nc.sync.dma_start(out=brow[1:2, :], in_=blo)

            # ---- weights: quantize + transpose, resident (all on DVE) ----
            wcfg = dict(
                cfg, reduce="v", mult="v", clamp="v", round="v", scalemult="v"
            )
            wqT = []
            wq_tiles = []
            for ob in range(OB):
                w_raw = wsb.tile([P, K], f32, tag="w_raw", bufs=OB)
                nc.sync.dma_start(out=w_raw, in_=w_d[ob * P : (ob + 1) * P, :])
                wq = wsb.tile([P, K], bf16, tag="wq", bufs=OB)
                uw, scw = _quant_a(nc, pools, w_raw, K, w_bit, w_sz, wcfg)
                _quant_b(nc, pools, uw, scw, K, w_bit, w_sz, wq, wcfg)
                wq_tiles.append(wq)
            for cp in range(KC // 2):
                ptw = psT.tile([P, 2, O], bf16, tag="ptT")
                for g in range(2):
                    ci = cp * 2 + g
                    for ob in range(OB):
                        nc.tensor.transpose(
                            ptw[:, g, ob * P : (ob + 1) * P],
                            wq_tiles[ob][:, ci * P : (ci + 1) * P],
                            ident,
                        )
                wt = wsb.tile([P, 2, O], bf16, tag=f"wqT{cp}")
                nc.scalar.copy(wt, ptw)
                wqT.extend([wt[:, 0, :], wt[:, 1, :]])

            # ---- software-pipelined main loop ----
            st = {}

            def dma_in(c):
                x_raw = xraw.tile([P, FB, K], f32, tag="x_raw")
                src = x_d[c * RPC : (c + 1) * RPC, :].rearrange(
                    "(f p) k -> p f k", p=P
                )
                nc.sync.dma_start(out=x_raw, in_=src)
                st[c] = {"x": x_raw}

            def quant_a(c):
                s = st[c]
                xt = s["x"].rearrange("p f k -> p (f k)")
                s["u"], s["scab"] = _quant_a(
                    nc, pools, xt, F, x_bit, x_sz, cfg, ci=c
                )

            def quant_b(c):
                s = st[c]
                xq = xqp.tile([P, F], bf16, tag="xq")
                _quant_b(
                    nc, pools, s["u"], s["scab"], F, x_bit, x_sz, xq, cfg, ci=c
                )
                s["xq"] = xq

            def mm_out(c):
                s = st.pop(c)
                xq_nat = s["xq"].rearrange("p (f c q) -> p f c q", f=FB, c=KC)
                ptTs = []
                xqTs = []
                for fp in range(FB // 2):
                    ptT = psT.tile([P, 2, K], bf16, tag="ptT")
                    for g in range(2):
                        f = fp * 2 + g
                        for ci in range(KC):
                            nc.tensor.transpose(
                                ptT[:, g, ci * P : (ci + 1) * P],
                                xq_nat[:, f, ci],
                                ident,
                            )
                    xqT = xqTp.tile([P, 2, K], bf16, tag="xqT")
                    if cfg["xqtcopy"] == "a":
                        nc.scalar.copy(xqT, ptT)
                    else:
                        _eng(nc, cfg["xqtcopy"], c).tensor_copy(out=xqT, in_=ptT)
                    ptTs.append(ptT)
                    xqTs.append(xqT)
                for fp in range(FB // 2):
                    xqT = xqTs[fp]
                    po = psO.tile([P, 2, O], f32, tag="po")
                    for g in range(2):
                        nc.tensor.matmul(
                            po[:, g, :], lhsT=ones2, rhs=brow, start=True, stop=False
                        )
                        for ci in range(KC):
                            nc.tensor.matmul(
                                po[:, g, :],
                                lhsT=xqT[:, g, ci * P : (ci + 1) * P],
                                rhs=wqT[ci],
                                start=False,
                                stop=(ci == KC - 1),
                            )
                    out_sb = osb.tile([P, 2, O], bf16, tag="out_sb")
                    if cfg["outcopy"] == "a":
                        nc.scalar.copy(out_sb, po)
                    else:
                        _eng(nc, cfg["outcopy"], c).tensor_copy(out=out_sb, in_=po)
                    r0 = c * RPC + fp * 2 * P
                    dst = o_d[r0 : r0 + 2 * P, :].rearrange("(f p) k -> p f k", p=P)
                    nc.sync.dma_start(out=dst, in_=out_sb)

            dma_in(0)
            quant_a(0)
            for c in range(n_chunks):
                if c + 1 < n_chunks:
                    dma_in(c + 1)
                if c + 1 < n_chunks:
                    quant_a(c + 1)
                quant_b(c)
                mm_out(c)
    nc.compile()
    return nc


def _get_program(nrows, K, O, x_bit, w_bit, x_sz, w_sz):
    key = (nrows, K, O, x_bit, w_bit, x_sz, w_sz)
    if key not in _CACHE:
        _CACHE[key] = _build(nrows, K, O, x_bit, w_bit, x_sz, w_sz)
    return _CACHE[key]


def kernel(input, weight, bias, i_bit, i_sz, w_bit, w_sz):
    from concourse.bass_utils import run_bass_kernel_spmd

    x = np.ascontiguousarray(np.asarray(input, dtype=np.float32))
    w = np.ascontiguousarray(np.asarray(weight, dtype=np.float32))
    b = np.ascontiguousarray(np.asarray(bias, dtype=np.float32)).reshape(1, -1)
    i_bit, i_sz, w_bit, w_sz = int(i_bit), int(i_sz), int(w_bit), int(w_sz)

    N, K = x.shape
    O = w.shape[0]
    assert N % N_CORES == 0
    shard = N // N_CORES

    nc = _get_program(shard, K, O, i_bit, w_bit, i_sz, w_sz)
    in_maps = [
        {"x": x[i * shard : (i + 1) * shard], "w": w, "b": b} for i in range(N_CORES)
    ]
    res = run_bass_kernel_spmd(nc, in_maps, list(range(N_CORES)))
    out = np.concatenate(
        [np.asarray(r["out"]).astype(np.float32) for r in res.results], axis=0
    )
    return out


# revision 37
# speedup vs baseline: 1.2122x; 1.2122x over previous
"""BFP-quantized linear layer (BFLinear) for Trainium2, 8-core data-parallel.

Computes: out = bfp_q(x, 8, 16) @ bfp_q(w, 8, 16).T + bias
  where bfp_q groups 16 contiguous elements along the feature axis, shares
  exponent e = floor(log2(max|g|)), rounds mantissas to `bit` bits (RNE) and
  clips to [-2^(bit-1), 2^(bit-1)-1].

Math on-device (bit-exact vs the jax reference, up to matmul accumulation):
  gmax  = max|group|                       (DVE reduce, abs)
  gmc   = max(gmax, FLT_MIN)
  recipB= 2^(bit-1-e)  (bit tricks: ((bits&EM)^EM) + (bit-2)<<23)
  sca   = 2^(e-(bit-1)) = (bits&EM as float) * 2^-(bit-1)
  v     = x * recipB                       (TT, exact pow2 scaling, Pool)
  u     = clamp(v, lo, hi)                 (TS min/max, Pool)
  t     = (u + C) + (-C) -> bf16 ints      (TS add/add 2x on DVE; C=1.5*2^23
                                            forces RNE at integer granularity)
  xq    = t * bf16(sca)                    (TT bf16, exact, DVE)
Then out = xq @ wq.T + bias via bf16 TensorE matmuls accumulated in fp32 PSUM;
bias seeded into PSUM by a K=2 bf16 matmul (ones x [bias_hi; bias_lo]).
Output is written bf16 (error <= 2^-9 relative, well within tolerance) and
upcast to f32 on host, halving output HBM traffic.

Sharding: rows of x split evenly across 8 NeuronCores; weight/bias replicated.
Quantization groups lie along K (feature) so row sharding never splits one.

Scheduling: engine queues execute in order, so the emission is software-
pipelined — input DMA runs two chunks ahead and the reduce/smalls/mult/clamp
stage one chunk ahead of the round/scale stage; this keeps the DVE from
head-of-line blocking on the Pool's clamp and keeps PE continuously fed
(holding its fast p-state).

Hardware op-shape rules learned from traces (violating any costs 2-25x):
  - scalar_tensor_tensor is DVE-only; tensor_scalar on Pool only with min/max;
    no subtract-immediates or negative int immediates anywhere on TS;
    f32->bf16 TS writes only via the (add,add) dual; casts on ACT, not DVE.
"""

import os
import sys

import numpy as np

for _p in ("/opt/trn_rl_repo",):
    if _p not in sys.path and os.path.isdir(_p):
        sys.path.append(_p)

N_CORES = 8

# engine per stage: 'v' DVE, 'g' GPSIMD/Pool, 'a' ACT/scalar
ENG_CFG = {
    "reduce": "v",      # gmax group absmax (DVE only: gpsimd can't reduce X)
    "smalls": "v",      # [P, G] bit-trick ops
    "mult": "v",        # v = x * recipB  (TT f32; Pool TT is 2x slower and
                        # contends with DVE on the shared SBUF port)
    "clamp": "g",       # u = clamp(v, lo, hi)  (TS min/max — Pool-proven)
    "round": "v",       # t = (u + C) + -C -> bf16  (TS add/add, 2x on DVE)
    "scalemult": "v",   # xq = t * scab  (TT bf16, DVE)
    "xqtcopy": "a",     # PSUM->SBUF copy of transposed xq
    "outcopy": "a",     # PSUM->SBUF copy of out (f32->bf16)
}

_CACHE = {}


def _eng(nc, which, idx=0):
    s = {"v": nc.vector, "g": nc.gpsimd, "a": nc.scalar}
    return s[which[idx % len(which)]]


def _bcast_group_ap(t, G, sz):
    """AP reading tile t[P, G] as [P, G, sz] with the last dim broadcast."""
    import concourse.bass as bass

    ap = t.ap.copy()
    ap.append([0, sz])
    return bass.AP(tensor=t.tensor, offset=t.offset, ap=ap)


def _quant_a(nc, pools, xt, F, bit, sz, cfg, ci=0):
    """Stage A: group stats + normalize + clamp. Returns (u, scab)."""
    import concourse.mybir as mybir

    f32 = mybir.dt.float32
    i32 = mybir.dt.int32
    bf16 = mybir.dt.bfloat16
    P = 128
    G = F // sz
    qmax = float(2 ** (bit - 1) - 1)
    hi = float(np.nextafter(np.float32(qmax + 0.5), np.float32(0.0)))
    lo = float(np.nextafter(np.float32(-qmax - 1.5), np.float32(0.0)))
    FLT_MIN = float(2.0**-119)
    EXPMASK = 0x7F800000

    er = _eng(nc, cfg["reduce"], ci)
    es = _eng(nc, cfg["smalls"], ci)
    em = _eng(nc, cfg["mult"], ci)
    eu = _eng(nc, cfg["clamp"], ci)

    gmax = pools["sml"].tile([P, G], f32, tag="gmax")
    er.tensor_reduce(
        out=gmax,
        in_=xt.rearrange("p (g s) -> p g s", s=sz),
        axis=mybir.AxisListType.X,
        op=mybir.AluOpType.max,
        apply_absolute_value=True,
    )
    gmc = pools["sml"].tile([P, G], f32, tag="gmc")
    es.tensor_scalar(
        out=gmc, in0=gmax, scalar1=FLT_MIN, scalar2=None, op0=mybir.AluOpType.max
    )
    recip = pools["sml"].tile([P, G], i32, tag="recip")
    es.tensor_scalar(
        out=recip,
        in0=gmc.bitcast(i32),
        scalar1=EXPMASK,
        scalar2=EXPMASK,
        op0=mybir.AluOpType.bitwise_and,
        op1=mybir.AluOpType.bitwise_xor,
    )
    recipB = pools["sml"].tile([P, G], i32, tag="recipB")
    es.tensor_scalar(
        out=recipB,
        in0=recip,
        scalar1=(bit - 2) << 23,
        scalar2=None,
        op0=mybir.AluOpType.add,
    )
    pow2e = pools["sml"].tile([P, G], i32, tag="pow2e")
    es.tensor_scalar(
        out=pow2e,
        in0=gmc.bitcast(i32),
        scalar1=EXPMASK,
        scalar2=None,
        op0=mybir.AluOpType.bitwise_and,
    )
    # sca = pow2e * 2^-(bit-1) as float mult (exact; int subtract / negative
    # int-add immediates fall off the DVE fast path: 3.6-4.4us for [128,128])
    sca = pools["sml"].tile([P, G], f32, tag="sca")
    es.tensor_scalar(
        out=sca,
        in0=pow2e.bitcast(f32),
        scalar1=float(2.0 ** (-(bit - 1))),
        scalar2=None,
        op0=mybir.AluOpType.mult,
    )
    # f32->bf16 cast: ~300ns on ACT vs ~1us CAST on DVE
    scab = pools["sml"].tile([P, G], bf16, tag="scab")
    nc.scalar.copy(scab, sca)

    v = pools["v"].tile([P, F], f32, tag="v")
    em.tensor_tensor(
        out=v,
        in0=xt,
        in1=_bcast_group_ap(recipB.bitcast(f32), G, sz),
        op=mybir.AluOpType.mult,
    )
    u = pools["u"].tile([P, F], f32, tag="u")
    eu.tensor_scalar(
        out=u,
        in0=v,
        scalar1=hi,
        scalar2=lo,
        op0=mybir.AluOpType.min,
        op1=mybir.AluOpType.max,
    )
    return u, scab


def _quant_b(nc, pools, u, scab, F, bit, sz, out_bf16, cfg, ci=0):
    """Stage B: RNE round to bf16 ints, then exact scale-back (bf16 TT)."""
    import concourse.mybir as mybir

    bf16 = mybir.dt.bfloat16
    P = 128
    G = F // sz
    C = float(np.float32(1.5 * 2.0**23))

    et = _eng(nc, cfg["round"], ci)
    ex = _eng(nc, cfg["scalemult"], ci)

    t = pools["t"].tile([P, F], bf16, tag="t")
    et.tensor_scalar(
        out=t,
        in0=u,
        scalar1=C,
        scalar2=-C,
        op0=mybir.AluOpType.add,
        op1=mybir.AluOpType.add,
    )
    ex.tensor_tensor(
        out=out_bf16,
        in0=t,
        in1=_bcast_group_ap(scab, G, sz),
        op=mybir.AluOpType.mult,
    )


def _build(nrows, K, O, x_bit, w_bit, x_sz, w_sz, cfg=None):
    import concourse.bacc as bacc
    import concourse.bass as bass  # noqa: F401
    import concourse.mybir as mybir
    import concourse.tile as tile
    from concourse.masks import make_identity

    cfg = dict(ENG_CFG, **(cfg or {}))
    f32 = mybir.dt.float32
    bf16 = mybir.dt.bfloat16

    P = 128
    RPC = 512  # rows per chunk
    assert nrows % RPC == 0
    n_chunks = nrows // RPC
    FB = RPC // P  # row-blocks per chunk (8)
    F = FB * K  # free columns per chunk
    KC = K // P  # k-chunks (4)
    OB = O // P  # o-blocks (4)

    nc = bacc.Bacc("TRN2", debug=False)
    x_d = nc.dram_tensor("x", (nrows, K), f32, kind="ExternalInput").ap()
    w_d = nc.dram_tensor("w", (O, K), f32, kind="ExternalInput").ap()
    b_d = nc.dram_tensor("b", (1, O), f32, kind="ExternalInput").ap()
    o_d = nc.dram_tensor("out", (nrows, O), bf16, kind="ExternalOutput").ap()

    with tile.TileContext(nc) as tc:
        with (
            tc.tile_pool(name="const", bufs=1) as constp,
            tc.tile_pool(name="wsb", bufs=1) as wsb,
            tc.tile_pool(name="xraw", bufs=3) as xraw,
            tc.tile_pool(name="sml", bufs=4) as sml,
            tc.tile_pool(name="v", bufs=3) as vp,
            tc.tile_pool(name="u", bufs=3) as up,
            tc.tile_pool(name="t", bufs=3) as tp,
            tc.tile_pool(name="xq", bufs=3) as xqp,
            tc.tile_pool(name="xqT", bufs=4) as xqTp,
            tc.tile_pool(name="osb", bufs=4) as osb,
            tc.tile_pool(name="psT", bufs=2, space="PSUM") as psT,
            tc.tile_pool(name="psO", bufs=3, space="PSUM") as psO,
        ):
            pools = {"sml": sml, "v": vp, "u": up, "t": tp}

            ident = constp.tile([P, P], bf16)
            make_identity(nc, ident)
            ones2 = constp.tile([2, P], bf16)
            nc.vector.memset(ones2, 1.0)
            bias_sb = constp.tile([1, O], f32)
            nc.sync.dma_start(out=bias_sb, in_=b_d)
            # bias split into bf16 hi + lo so a K=2 bf16 matmul seeds PSUM
            # with fp32-accurate bias (error ~2^-17 relative)
            bhi = constp.tile([1, O], bf16)
            nc.vector.tensor_copy(out=bhi, in_=bias_sb)
            bhi32 = constp.tile([1, O], f32)
            nc.vector.tensor_copy(out=bhi32, in_=bhi)
            blo32 = constp.tile([1, O], f32)
            nc.vector.tensor_tensor(
                out=blo32, in0=bias_sb, in1=bhi32, op=mybir.AluOpType.subtract
            )
            blo = constp.tile([1, O], bf16)
            nc.vector.tensor_copy(out=blo, in_=blo32)
            brow = constp.tile([2, O], bf16)
            nc.sync.dma_start(out=brow[0:1, :], in_=bhi)
            nc.sync.dma_start(out=brow[1:2, :], in_=blo)

            # ---- weights: quantize + transpose, resident (all on DVE) ----
            wcfg = dict(
                cfg, reduce="v", mult="v", clamp="v", round="v", scalemult="v"
            )
            wqT = []
            wq_tiles = []
            for ob in range(OB):
                w_raw = wsb.tile([P, K], f32, tag="w_raw", bufs=OB)
                nc.sync.dma_start(out=w_raw, in_=w_d[ob * P : (ob + 1) * P, :])
                wq = wsb.tile([P, K], bf16, tag="wq", bufs=OB)
                uw, scw = _quant_a(nc, pools, w_raw, K, w_bit, w_sz, wcfg)
                _quant_b(nc, pools, uw, scw, K, w_bit, w_sz, wq, wcfg)
                wq_tiles.append(wq)
            for cp in range(KC // 2):
                ptw = psT.tile([P, 2, O], bf16, tag="ptT")
                for g in range(2):
                    ci = cp * 2 + g
                    for ob in range(OB):
                        nc.tensor.transpose(
                            ptw[:, g, ob * P : (ob + 1) * P],
                            wq_tiles[ob][:, ci * P : (ci + 1) * P],
                            ident,
                        )
                wt = wsb.tile([P, 2, O], bf16, tag=f"wqT{cp}")
                nc.scalar.copy(wt, ptw)
                wqT.extend([wt[:, 0, :], wt[:, 1, :]])

            # ---- software-pipelined main loop ----
            st = {}

            def dma_in(c):
                x_raw = xraw.tile([P, FB, K], f32, tag="x_raw")
                src = x_d[c * RPC : (c + 1) * RPC, :].rearrange(
                    "(f p) k -> p f k", p=P
                )
                nc.sync.dma_start(out=x_raw, in_=src)
                st[c] = {"x": x_raw}

            def quant_a(c):
                s = st[c]
                xt = s["x"].rearrange("p f k -> p (f k)")
                s["u"], s["scab"] = _quant_a(
                    nc, pools, xt, F, x_bit, x_sz, cfg, ci=c
                )

            def quant_b(c):
                s = st[c]
                xq = xqp.tile([P, F], bf16, tag="xq")
                _quant_b(
                    nc, pools, s["u"], s["scab"], F, x_bit, x_sz, xq, cfg, ci=c
                )
                s["xq"] = xq

            def mm_out(c):
                s = st.pop(c)
                xq_nat = s["xq"].rearrange("p (f c q) -> p f c q", f=FB, c=KC)
                ptTs = []
                xqTs = []
                for fp in range(FB // 2):
                    ptT = psT.tile([P, 2, K], bf16, tag="ptT")
                    for g in range(2):
                        f = fp * 2 + g
                        for ci in range(KC):
                            nc.tensor.transpose(
                                ptT[:, g, ci * P : (ci + 1) * P],
                                xq_nat[:, f, ci],
                                ident,
                            )
                    xqT = xqTp.tile([P, 2, K], bf16, tag="xqT")
                    if cfg["xqtcopy"] == "a":
                        nc.scalar.copy(xqT, ptT)
                    else:
                        _eng(nc, cfg["xqtcopy"], c).tensor_copy(out=xqT, in_=ptT)
                    ptTs.append(ptT)
                    xqTs.append(xqT)
                for fp in range(FB // 2):
                    xqT = xqTs[fp]
                    po = psO.tile([P, 2, O], f32, tag="po")
                    for g in range(2):
                        nc.tensor.matmul(
                            po[:, g, :], lhsT=ones2, rhs=brow, start=True, stop=False
                        )
                        for ci in range(KC):
                            nc.tensor.matmul(
                                po[:, g, :],
                                lhsT=xqT[:, g, ci * P : (ci + 1) * P],
                                rhs=wqT[ci],
                                start=False,
                                stop=(ci == KC - 1),
                            )
                    out_sb = osb.tile([P, 2, O], bf16, tag="out_sb")
                    if cfg["outcopy"] == "a":
                        nc.scalar.copy(out_sb, po)
                    else:
                        _eng(nc, cfg["outcopy"], c).tensor_copy(out=out_sb, in_=po)
                    r0 = c * RPC + fp * 2 * P
                    dst = o_d[r0 : r0 + 2 * P, :].rearrange("(f p) k -> p f k", p=P)
                    nc.sync.dma_start(out=dst, in_=out_sb)

            dma_in(0)
            if n_chunks > 1:
                dma_in(1)
            quant_a(0)
            for c in range(n_chunks):
                if c + 2 < n_chunks:
                    dma_in(c + 2)
                if c + 1 < n_chunks:
                    quant_a(c + 1)
                quant_b(c)
                mm_out(c)
    nc.compile()
    return nc


def _get_program(nrows, K, O, x_bit, w_bit, x_sz, w_sz):
    key = (nrows, K, O, x_bit, w_bit, x_sz, w_sz)
    if key not in _CACHE:
        _CACHE[key] = _build(nrows, K, O, x_bit, w_bit, x_sz, w_sz)
    return _CACHE[key]


def kernel(input, weight, bias, i_bit, i_sz, w_bit, w_sz):
    from concourse.bass_utils import run_bass_kernel_spmd

    x = np.ascontiguousarray(np.asarray(input, dtype=np.float32))
    w = np.ascontiguousarray(np.asarray(weight, dtype=np.float32))
    b = np.ascontiguousarray(np.asarray(bias, dtype=np.float32)).reshape(1, -1)
    i_bit, i_sz, w_bit, w_sz = int(i_bit), int(i_sz), int(w_bit), int(w_sz)

    N, K = x.shape
    O = w.shape[0]
    assert N % N_CORES == 0
    shard = N // N_CORES

    nc = _get_program(shard, K, O, i_bit, w_bit, i_sz, w_sz)
    in_maps = [
        {"x": x[i * shard : (i + 1) * shard], "w": w, "b": b} for i in range(N_CORES)
    ]
    res = run_bass_kernel_spmd(nc, in_maps, list(range(N_CORES)))
    out = np.concatenate(
        [np.asarray(r["out"]).astype(np.float32) for r in res.results], axis=0
    )
    return out
